# revision 1
# baseline (speedup 1.0000x reference)
"""CRF negative-log-likelihood kernel for Trainium2 (8 NeuronCores, batch-sharded).

Algorithm:
  - t2 = embedding @ fc_w computed on-device, vocab-sharded across cores (launch 1).
  - Main kernel (launch 2, batch-sharded 8 rows/core): indirect-DMA gather of
    t2 rows (16 floats/token instead of 128 -> 8x less gather traffic), PE-block
    transposes into class-on-partition layout, numerator via one-hot matmul +
    fused multiply-reduce, and a segmented forward/backward scan (L=16 steps,
    S=256 segments batched on the free dim) in linear space.
  - Host (float64, O(B*S*C) work): rank-1 junction chain across segments,
    exact partial segment for each row's ragged tail, final scalar assembly.
"""
import sys
sys.path.insert(0, "/opt/trn_rl_repo")
import numpy as np
from contextlib import ExitStack

import concourse.bass as bass
import concourse.bacc as bacc_mod
import concourse.mybir as mybir
import concourse.tile as tile
from concourse.masks import make_identity
from concourse.bass_utils import run_bass_kernel_spmd

F32 = mybir.dt.float32
I32 = mybir.dt.int32

V, E, C = 50257, 128, 16
B, T = 64, 4096
L, S = 16, 256
VPAD = 51200
VSH = VPAD // 8
BL = 8
NCHUNK = 8
CHW = T // NCHUNK
NCORES = 8

LAST_EXEC_NS = {}
_TRACE = False
_CACHE = {}


def build_t2_kernel():
    nc = bacc_mod.Bacc()
    emb_s = nc.dram_tensor("emb_s", [VSH, E], F32, kind="ExternalInput")
    fc_w = nc.dram_tensor("fc_w", [E, C], F32, kind="ExternalInput")
    t2_s = nc.dram_tensor("t2_s", [VSH, C], F32, kind="ExternalOutput")

    ntile = VSH // 128
    with ExitStack() as ctx:
        tc = ctx.enter_context(tile.TileContext(nc))
        singles = ctx.enter_context(tc.tile_pool(name="singles", bufs=1))
        psum = ctx.enter_context(tc.tile_pool(name="psum", bufs=4, space="PSUM"))

        fcw_sb = singles.tile([E, C], F32)
        nc.sync.dma_start(out=fcw_sb[:], in_=fc_w[:])
        ident = singles.tile([128, 128], F32)
        make_identity(nc, ident[:])

        # one DMA: all of emb_s, 50 blocks of (128,128) side by side
        EMB = singles.tile([128, VSH], F32)
        nc.sync.dma_start(
            out=EMB[:],
            in_=bass.AP(tensor=emb_s.handle if hasattr(emb_s, "handle") else emb_s[:].tensor,
                        offset=0, ap=[[E, 128], [128 * E, ntile], [1, E]]))
        ET = singles.tile([128, VSH], F32)
        T2 = singles.tile([128, ntile * C], F32)
        for i in range(ntile):
            psT = psum.tile([128, 128], F32, tag="pt")
            nc.tensor.transpose(psT[:], EMB[:, i * 128:(i + 1) * 128], ident[:])
            nc.vector.tensor_copy(ET[:, i * 128:(i + 1) * 128], psT[:])
        for i in range(ntile):
            ps2 = psum.tile([128, C], F32, tag="p2")
            nc.tensor.matmul(ps2[:], lhsT=ET[:, i * 128:(i + 1) * 128], rhs=fcw_sb[:],
                             start=True, stop=True)
            nc.vector.tensor_copy(T2[:, i * C:(i + 1) * C], ps2[:])
        # one DMA out: (128, ntile*C) -> t2_s (VSH, C); dst dims (r, i, j)
        nc.sync.dma_start(
            out=bass.AP(tensor=t2_s[:].tensor, offset=0,
                        ap=[[C, 128], [128 * C, ntile], [1, C]]),
            in_=T2[:])
    return nc


def _tokgather_ap(base_ap, thi):
    """Indirect-gather dest over TM tile (128, T): partition = t%128, free =
    (t//128)*128 + b*16 + j; token (b,t)'s 16 floats land contiguously.
    Partition-first enumeration (t_lo, b, j) matches the x_t index order."""
    Fd = base_ap.ap[1][1]
    return bass.AP(tensor=base_ap.tensor, offset=base_ap.offset + thi * 128,
                   ap=[[Fd, 128], [16, BL], [1, 16]])


def _strided(base_ap, k, step, count):
    return bass.AP(tensor=base_ap.tensor, offset=base_ap.offset + k,
                   ap=[base_ap.ap[0], [step, count]])


def build_main_kernel():
    nc = bacc_mod.Bacc()
    x_t = nc.dram_tensor("x_t", [128, T // 128 * BL], I32, kind="ExternalInput")
    tags_f = nc.dram_tensor("tags_f", [BL, T], F32, kind="ExternalInput")
    t2 = nc.dram_tensor("t2", [VPAD, C], F32, kind="ExternalInput")
    blockP = nc.dram_tensor("blockP", [128, 128], F32, kind="ExternalInput")
    blockPT = nc.dram_tensor("blockPT", [128, 128], F32, kind="ExternalInput")
    blockTN = nc.dram_tensor("blockTN", [128, 128], F32, kind="ExternalInput")
    bcast8 = nc.dram_tensor("bcast8", [BL, 128], F32, kind="ExternalInput")
    iota_rep = nc.dram_tensor("iota_rep", [128, CHW], F32, kind="ExternalInput")
    sadj = nc.dram_tensor("sadj", [128, 1], F32, kind="ExternalInput")

    r_out = nc.dram_tensor("r_out", [128, S], F32, kind="ExternalOutput")
    d_out = nc.dram_tensor("d_out", [128, S], F32, kind="ExternalOutput")
    num_out = nc.dram_tensor("num_out", [128, 2 * NCHUNK], F32, kind="ExternalOutput")

    with ExitStack() as ctx:
        tc = ctx.enter_context(tile.TileContext(nc))
        singles = ctx.enter_context(tc.tile_pool(name="singles", bufs=1))
        big = ctx.enter_context(tc.tile_pool(name="big", bufs=1))
        scratch = ctx.enter_context(tc.tile_pool(name="scratch", bufs=3))
        psum = ctx.enter_context(tc.tile_pool(name="psum", bufs=2, space="PSUM"))
        psum2 = ctx.enter_context(tc.tile_pool(name="psum2", bufs=1, space="PSUM"))

        xt_sb = singles.tile([128, T // 128 * BL], I32)
        nc.sync.dma_start(out=xt_sb[:], in_=x_t[:])
        tagsf_sb = singles.tile([BL, T], F32)
        nc.sync.dma_start(out=tagsf_sb[:], in_=tags_f[:])
        blockP_sb = singles.tile([128, 128], F32)
        nc.sync.dma_start(out=blockP_sb[:], in_=blockP[:])
        blockPT_sb = singles.tile([128, 128], F32)
        nc.sync.dma_start(out=blockPT_sb[:], in_=blockPT[:])
        blockTN_sb = singles.tile([128, 128], F32)
        nc.sync.dma_start(out=blockTN_sb[:], in_=blockTN[:])
        bcast8_sb = singles.tile([BL, 128], F32)
        nc.sync.dma_start(out=bcast8_sb[:], in_=bcast8[:])
        iotar_sb = singles.tile([128, CHW], F32)
        nc.sync.dma_start(out=iotar_sb[:], in_=iota_rep[:])
        sadj_sb = singles.tile([128, 1], F32)
        nc.sync.dma_start(out=sadj_sb[:], in_=sadj[:])

        TM = big.tile([128, T], F32)
        G = big.tile([128, T], F32)
        EXPG = big.tile([128, T], F32)
        W_ext = big.tile([128, T + 4], F32)
        num_sb = singles.tile([128, 2 * NCHUNK], F32)
        ident = singles.tile([128, 128], F32)
        make_identity(nc, ident[:])

        nc.vector.memset(W_ext[:, 0:1], 0.0)
        nc.vector.memset(num_sb[:], 0.0)

        TMap = TM[:]
        EXPGap = EXPG[:]

        # --- gather (token-major) + transpose blocks into G + exp ---
        for c in range(NCHUNK):
            c0 = c * CHW
            nthi = CHW // 128
            for th in range(c * nthi, (c + 1) * nthi):
                for bb in range(BL):
                    cc = th * BL + bb
                    nc.gpsimd.indirect_dma_start(
                        out=TM[:, cc * 16:(cc + 1) * 16],
                        out_offset=None,
                        in_=t2[:],
                        in_offset=bass.IndirectOffsetOnAxis(
                            ap=xt_sb[:, cc:cc + 1], axis=0),
                    )
                psT = psum.tile([128, 128], F32, tag="psT")
                nc.tensor.transpose(psT[:], TM[:, th * 128:(th + 1) * 128], ident[:])
                nc.any.tensor_copy(G[:, th * 128:(th + 1) * 128], psT[:])
            nc.scalar.activation(EXPG[:, c0:c0 + CHW], G[:, c0:c0 + CHW],
                                 mybir.ActivationFunctionType.Exp)
        nc.vector.tensor_mul(EXPG[:, 0:1], EXPG[:, 0:1], sadj_sb[:])

        # --- numerator ---
        for c in range(NCHUNK):
            c0 = c * CHW
            psA = psum.tile([128, CHW], F32, tag="ps")
            nc.tensor.matmul(psA[:], lhsT=bcast8_sb[:],
                             rhs=tagsf_sb[:, c0:c0 + CHW], start=True, stop=True)
            nc.vector.tensor_tensor(out=W_ext[:, 1 + c0:1 + c0 + CHW], in0=psA[:],
                                    in1=iotar_sb[:], op=mybir.AluOpType.is_equal)
        for c in range(NCHUNK):
            c0 = c * CHW
            psY = psum.tile([128, CHW], F32, tag="ps")
            nc.tensor.matmul(psY[:], lhsT=blockTN_sb[:],
                             rhs=W_ext[:, c0:c0 + CHW], start=True, stop=True)
            scr = scratch.tile([128, CHW], F32, tag="scr")
            nc.vector.tensor_add(scr[:], G[:, c0:c0 + CHW], psY[:])
            scr2 = scratch.tile([128, CHW], F32, tag="scr2")
            nc.vector.tensor_mul(scr2[:], scr[:], W_ext[:, 1 + c0:1 + c0 + CHW])
            nc.vector.reduce_sum(out=num_sb[:, c:c + 1], in_=scr2[:],
                                 axis=mybir.AxisListType.X)

        # --- scans ---
        r_sb = big.tile([128, S], F32)
        nc.vector.memset(r_sb[:], 1.0)
        for k in range(L):
            psR = psum2.tile([128, S], F32, tag="psR")
            nc.tensor.matmul(psR[:], lhsT=blockP_sb[:], rhs=r_sb[:],
                             start=True, stop=True)
            nc.vector.tensor_mul(r_sb[:], psR[:], _strided(EXPGap, k, L, S))

        d_sb = big.tile([128, S], F32)
        nc.vector.tensor_copy(d_sb[:], _strided(EXPGap, L - 1, L, S))
        for k in range(L - 2, -1, -1):
            psD = psum2.tile([128, S], F32, tag="psD")
            nc.tensor.matmul(psD[:], lhsT=blockPT_sb[:], rhs=d_sb[:],
                             start=True, stop=True)
            nc.vector.tensor_mul(d_sb[:], psD[:], _strided(EXPGap, k, L, S))

        nc.sync.dma_start(out=r_out[:], in_=r_sb[:])
        nc.sync.dma_start(out=d_out[:], in_=d_sb[:])
        nc.sync.dma_start(out=num_out[:], in_=num_sb[:])
    return nc


def _host_prep(embedding, fc_w, fc_b, trans, start):
    emb_pad = np.zeros((VPAD, E), np.float32)
    emb_pad[:V] = embedding
    P_eff64 = np.exp(trans.astype(np.float64) + fc_b[None, :].astype(np.float64))
    colsum = P_eff64.sum(0)
    start_adj = (np.exp(start.astype(np.float64) + fc_b) / colsum).astype(np.float32)
    trans_n = (trans + fc_b[None, :]).astype(np.float32)
    P_eff32 = P_eff64.astype(np.float32)

    eye8 = np.eye(BL, dtype=np.float32)
    return dict(
        emb_pad=emb_pad,
        P_eff=P_eff64,
        blockP=np.ascontiguousarray(np.kron(eye8, P_eff32)),
        blockPT=np.ascontiguousarray(np.kron(eye8, P_eff32.T.copy())),
        blockTN=np.ascontiguousarray(np.kron(eye8, trans_n)),
        bcast8=np.ascontiguousarray(np.kron(eye8, np.ones((1, C), np.float32))),
        iota_rep=np.ascontiguousarray(np.tile(np.tile(np.arange(C, dtype=np.float32), BL)[:, None], (1, CHW))),
        sadj=np.ascontiguousarray(np.tile(start_adj, BL)[:, None]),
    )


LAST_RESULTS = {}


def _run(nc, in_maps, label):
    res = run_bass_kernel_spmd(nc, in_maps, core_ids=list(range(NCORES)),
                               trace=_TRACE)
    if res.exec_time_ns is not None:
        LAST_EXEC_NS[label] = res.exec_time_ns
    LAST_RESULTS[label] = res
    return res.results


def kernel(x, tags, embedding, fc_w, fc_b, start_transitions, end_transitions,
           transitions):
    x = np.asarray(x, np.int32)
    tags = np.asarray(tags, np.int32)
    embedding = np.asarray(embedding, np.float32)
    fc_w = np.asarray(fc_w, np.float32)
    fc_b = np.asarray(fc_b, np.float32)
    trans = np.asarray(transitions, np.float32)
    start = np.asarray(start_transitions, np.float32)
    end = np.asarray(end_transitions, np.float32)

    prep = _host_prep(embedding, fc_w, fc_b, trans, start)

    if "t2" not in _CACHE:
        nc1 = build_t2_kernel()
        nc1.finalize()
        _CACHE["t2"] = nc1
    if "main" not in _CACHE:
        nc2 = build_main_kernel()
        nc2.finalize()
        _CACHE["main"] = nc2

    # ---- launch 1: t2 = emb_pad @ fc_w, vocab-sharded ----
    in1 = [{"emb_s": np.ascontiguousarray(prep["emb_pad"][k * VSH:(k + 1) * VSH]),
            "fc_w": fc_w} for k in range(NCORES)]
    res1 = _run(_CACHE["t2"], in1, "t2")
    t2_full = np.concatenate([res1[k]["t2_s"] for k in range(NCORES)], axis=0)
    t2_full = np.ascontiguousarray(t2_full, dtype=np.float32)

    # ---- launch 2: main kernel, batch-sharded ----
    tags_m = np.where(x != 0, tags, C).astype(np.float32)
    in2 = []
    for k in range(NCORES):
        sl = slice(k * BL, (k + 1) * BL)
        xt = x[sl].reshape(BL, T // 128, 128).transpose(2, 1, 0) \
                  .reshape(128, T // 128 * BL)
        in2.append({
            "x_t": np.ascontiguousarray(xt),
            "tags_f": np.ascontiguousarray(tags_m[sl]),
            "t2": t2_full,
            "blockP": prep["blockP"], "blockPT": prep["blockPT"],
            "blockTN": prep["blockTN"], "bcast8": prep["bcast8"],
            "iota_rep": prep["iota_rep"], "sadj": prep["sadj"],
        })
    res2 = _run(_CACHE["main"], in2, "main")

    # ---- host combine (float64) ----
    lengths = (x != 0).sum(1)
    start64 = start.astype(np.float64)
    end64 = end.astype(np.float64)
    fcb64 = fc_b.astype(np.float64)
    Pe = prep["P_eff"]
    t264 = t2_full.astype(np.float64)
    exp_end = np.exp(end64)
    total = 0.0
    for core in range(NCORES):
        num_p = np.asarray(res2[core]["num_out"], np.float64)
        r = np.asarray(res2[core]["r_out"], np.float64).reshape(BL, C, S)
        d = np.asarray(res2[core]["d_out"], np.float64).reshape(BL, C, S)
        for b in range(BL):
            gb = core * BL + b
            ln = int(lengths[gb])
            num = num_p[b * C:(b + 1) * C, :].sum()
            num += start64[tags[gb, 0]] + fcb64[tags[gb, 0]]
            num += end64[tags[gb, ln - 1]]
            sstar = (ln - 1) // L
            logZ = 0.0
            for s in range(1, sstar):
                c_s = Pe @ d[b, :, s]
                logZ += np.log(r[b, :, s - 1] @ c_s) - np.log(r[b, :, s].sum())
            alpha = r[b, :, sstar - 1].copy()
            for t in range(sstar * L, ln):
                w = np.exp(t264[x[gb, t]] + fcb64)
                alpha = (alpha @ Pe) * w
            logZ += np.log(alpha @ exp_end)
            total += -(num - logZ)
    return np.array(total, dtype=np.float32)



# revision 5
# speedup vs baseline: 7.1303x; 7.1303x over previous
"""CRF negative-log-likelihood kernel for Trainium2 (8 NeuronCores, batch-sharded).

Algorithm:
  - Launch 1 (vocab-sharded): t2 = embedding @ fc_w in bf16 from a
    host-pretransposed embedding (no on-device transposes), fp32 out.
  - Launch 2 (batch-sharded, 8 rows/core): 4 big indirect-DMA gathers
    (8192 offsets each) fetch 16 fp32 per token from t2; PE block-transposes
    to class-on-partition; exp on ACT (bf16 out); segmented linear-space
    forward/backward scan with L=4 steps x S=1024 segments in bf16.
    Tokens are host-permuted k-major so scan slices are contiguous.
  - Host (float64, vectorized): gold-path numerator from t2, rank-1 junction
    chain across segments, exact ragged-tail recompute, final assembly.
"""
import sys
sys.path.insert(0, "/opt/trn_rl_repo")
import numpy as np
import ml_dtypes
from contextlib import ExitStack

import concourse.bass as bass
import concourse.bacc as bacc_mod
import concourse.mybir as mybir
import concourse.tile as tile
from concourse.masks import make_identity
from concourse.bass_utils import run_bass_kernel_spmd

F32 = mybir.dt.float32
BF16 = mybir.dt.bfloat16
I32 = mybir.dt.int32
BF = ml_dtypes.bfloat16

V, E, C = 50257, 128, 16
B, T = 64, 4096
L, S = 4, 1024
VPAD = 51200
VSH = VPAD // 8
BL = 8
NCORES = 8
NG = 4                      # gather instructions in main kernel
GCOL = T // NG              # columns of TM per gather
H = 2                       # independent scan chains (halves of S)
SH = S // H

LAST_EXEC_NS = {}
_TRACE = False
_CACHE = {}
LAST_RESULTS = {}


def build_t2_kernel():
    nc = bacc_mod.Bacc()
    embT = nc.dram_tensor("embT", [E, VSH], BF16, kind="ExternalInput")
    fc_w = nc.dram_tensor("fc_w", [E, C], BF16, kind="ExternalInput")
    t2_s = nc.dram_tensor("t2_s", [VSH, C], F32, kind="ExternalOutput")

    nblk = VSH // 128        # 50 blocks of 128 vocab rows
    NCH = 5                  # input DMA chunks
    CHB = nblk // NCH        # blocks per chunk
    with ExitStack() as ctx:
        tc = ctx.enter_context(tile.TileContext(nc))
        singles = ctx.enter_context(tc.tile_pool(name="singles", bufs=1))
        psum = ctx.enter_context(tc.tile_pool(name="psum", bufs=2, space="PSUM"))

        fcw_sb = singles.tile([E, C], BF16)
        nc.sync.dma_start(out=fcw_sb[:], in_=fc_w[:])
        EMB = singles.tile([E, VSH], BF16)
        T2 = singles.tile([128, nblk * C], F32)
        for ch in range(NCH):
            c0 = ch * CHB * 128
            nc.sync.dma_start(out=EMB[:, c0:c0 + CHB * 128],
                              in_=embT[:, c0:c0 + CHB * 128])
        for ch in range(NCH):
            ps = psum.tile([128, CHB * C], F32, tag="ps")
            for i in range(CHB):
                blk = ch * CHB + i
                nc.tensor.matmul(ps[:, i * C:(i + 1) * C],
                                 lhsT=EMB[:, blk * 128:(blk + 1) * 128],
                                 rhs=fcw_sb[:], start=True, stop=True)
            nc.vector.tensor_copy(T2[:, ch * CHB * C:(ch + 1) * CHB * C], ps[:])
        # (128, nblk*C) -> t2_s (VSH, C); dst dims (p, i, j)
        nc.sync.dma_start(
            out=bass.AP(tensor=t2_s[:].tensor, offset=0,
                        ap=[[C, 128], [128 * C, nblk], [1, C]]),
            in_=T2[:])
    return nc


def build_main_kernel():
    nc = bacc_mod.Bacc()
    x_t = nc.dram_tensor("x_t", [128, T // 128 * BL], I32, kind="ExternalInput")
    t2 = nc.dram_tensor("t2", [VPAD, C], F32, kind="ExternalInput")
    blockP = nc.dram_tensor("blockP", [128, 128], BF16, kind="ExternalInput")
    blockPT = nc.dram_tensor("blockPT", [128, 128], BF16, kind="ExternalInput")
    colsum = nc.dram_tensor("colsum", [128, 1], F32, kind="ExternalInput")
    sadj = nc.dram_tensor("sadj", [128, 1], BF16, kind="ExternalInput")

    r_out = nc.dram_tensor("r_out", [128, S], BF16, kind="ExternalOutput")
    d_out = nc.dram_tensor("d_out", [128, S], BF16, kind="ExternalOutput")

    with ExitStack() as ctx:
        tc = ctx.enter_context(tile.TileContext(nc))
        singles = ctx.enter_context(tc.tile_pool(name="singles", bufs=1))
        big = ctx.enter_context(tc.tile_pool(name="big", bufs=1))
        psumT = ctx.enter_context(tc.tile_pool(name="psumT", bufs=2, space="PSUM"))
        psumS = ctx.enter_context(tc.tile_pool(name="psumS", bufs=1, space="PSUM"))

        xt_sb = singles.tile([128, T // 128 * BL], I32)
        nc.sync.dma_start(out=xt_sb[:], in_=x_t[:])
        blockP_sb = singles.tile([128, 128], BF16)
        nc.sync.dma_start(out=blockP_sb[:], in_=blockP[:])
        blockPT_sb = singles.tile([128, 128], BF16)
        nc.sync.dma_start(out=blockPT_sb[:], in_=blockPT[:])
        colsum_sb = singles.tile([128, 1], F32)
        nc.sync.dma_start(out=colsum_sb[:], in_=colsum[:])
        sadj_sb = singles.tile([128, 1], BF16)
        nc.sync.dma_start(out=sadj_sb[:], in_=sadj[:])

        TM = big.tile([128, T], F32)
        EXPG = big.tile([128, T], BF16)
        ident = singles.tile([128, 128], F32)
        make_identity(nc, ident[:])

        # --- gather (4 big indirect DMAs) + PE transpose + exp ---
        for g in range(NG):
            c0 = g * GCOL
            nc.gpsimd.indirect_dma_start(
                out=TM[:, c0:c0 + GCOL],
                out_offset=None,
                in_=t2[:],
                in_offset=bass.IndirectOffsetOnAxis(
                    ap=xt_sb[:, g * (GCOL // 16):(g + 1) * (GCOL // 16)], axis=0),
            )
            nthi = GCOL // 128
            for grp in range(nthi // 4):
                psT = psumT.tile([128, 512], F32, tag="psT")
                for q in range(4):
                    th = g * nthi + grp * 4 + q
                    nc.tensor.transpose(psT[:, q * 128:(q + 1) * 128],
                                        TM[:, th * 128:(th + 1) * 128], ident[:])
                base = c0 + grp * 512
                nc.scalar.activation(EXPG[:, base:base + 512], psT[:],
                                     mybir.ActivationFunctionType.Exp)
        nc.vector.tensor_mul(EXPG[:, 0:1], EXPG[:, 0:1], sadj_sb[:])

        # --- scans: L steps, S segments, split into H independent chains ---
        r_sb = big.tile([128, S], BF16)
        d_sb = big.tile([128, S], BF16)
        for h in range(H):
            a, b = h * SH, (h + 1) * SH
            nc.vector.tensor_scalar_mul(r_sb[:, a:b], EXPG[:, a:b], colsum_sb[:])
            nc.vector.tensor_copy(d_sb[:, a:b],
                                  EXPG[:, (L - 1) * S + a:(L - 1) * S + b])
        for step in range(1, L):
            kf = step
            kb = L - 1 - step
            for h in range(H):
                a, b = h * SH, (h + 1) * SH
                psR = psumS.tile([128, SH], F32, tag=f"psR{h}")
                nc.tensor.matmul(psR[:], lhsT=blockP_sb[:], rhs=r_sb[:, a:b],
                                 start=True, stop=True)
                nc.vector.tensor_mul(r_sb[:, a:b], psR[:],
                                     EXPG[:, kf * S + a:kf * S + b])
                psD = psumS.tile([128, SH], F32, tag=f"psD{h}")
                nc.tensor.matmul(psD[:], lhsT=blockPT_sb[:], rhs=d_sb[:, a:b],
                                 start=True, stop=True)
                nc.vector.tensor_mul(d_sb[:, a:b], psD[:],
                                     EXPG[:, kb * S + a:kb * S + b])

        nc.sync.dma_start(out=r_out[:], in_=r_sb[:])
        nc.sync.dma_start(out=d_out[:], in_=d_sb[:])
    return nc


def _host_prep(embedding, fc_w, fc_b, trans, start):
    emb_pad = np.zeros((VPAD, E), np.float32)
    emb_pad[:V] = embedding
    embT_pad = np.ascontiguousarray(emb_pad.T).astype(BF)
    P_eff64 = np.exp(trans.astype(np.float64) + fc_b[None, :].astype(np.float64))
    colsum64 = P_eff64.sum(0)
    start_adj = np.exp(start.astype(np.float64) + fc_b) / colsum64
    P_eff32 = P_eff64.astype(np.float32)

    eye8 = np.eye(BL, dtype=np.float32)
    return dict(
        embT_pad=embT_pad,
        P_eff=P_eff64,
        blockP=np.ascontiguousarray(np.kron(eye8, P_eff32)).astype(BF),
        blockPT=np.ascontiguousarray(np.kron(eye8, P_eff32.T.copy())).astype(BF),
        colsum=np.tile(colsum64, BL)[:, None].astype(np.float32),
        sadj=np.tile(start_adj, BL)[:, None].astype(BF),
    )


def _run(nc, in_maps, label):
    res = run_bass_kernel_spmd(nc, in_maps, core_ids=list(range(NCORES)),
                               trace=_TRACE)
    if res.exec_time_ns is not None:
        LAST_EXEC_NS[label] = res.exec_time_ns
    LAST_RESULTS[label] = res
    return res.results


def kernel(x, tags, embedding, fc_w, fc_b, start_transitions, end_transitions,
           transitions):
    x = np.asarray(x, np.int32)
    tags = np.asarray(tags, np.int32)
    embedding = np.asarray(embedding, np.float32)
    fc_w = np.asarray(fc_w, np.float32)
    fc_b = np.asarray(fc_b, np.float32)
    trans = np.asarray(transitions, np.float32)
    start = np.asarray(start_transitions, np.float32)
    end = np.asarray(end_transitions, np.float32)

    prep = _host_prep(embedding, fc_w, fc_b, trans, start)

    if "t2" not in _CACHE:
        nc1 = build_t2_kernel()
        nc1.finalize()
        _CACHE["t2"] = nc1
    if "main" not in _CACHE:
        nc2 = build_main_kernel()
        nc2.finalize()
        _CACHE["main"] = nc2

    # ---- launch 1: t2 = emb_pad @ fc_w, vocab-sharded ----
    fcw_bf = fc_w.astype(BF)
    in1 = [{"embT": np.ascontiguousarray(prep["embT_pad"][:, k * VSH:(k + 1) * VSH]),
            "fc_w": fcw_bf} for k in range(NCORES)]
    res1 = _run(_CACHE["t2"], in1, "t2")
    t2_full = np.concatenate([res1[k]["t2_s"] for k in range(NCORES)], axis=0)
    t2_full = np.ascontiguousarray(t2_full, dtype=np.float32)

    # ---- launch 2: main kernel, batch-sharded ----
    # token permutation: G column (th*128+p) holds token t = (col%S)*L + col//S
    cols = np.arange(T)
    tperm = (cols % S) * L + cols // S
    xp = x[:, tperm]
    in2 = []
    for k in range(NCORES):
        sl = slice(k * BL, (k + 1) * BL)
        xt = xp[sl].reshape(BL, T // 128, 128).transpose(2, 1, 0) \
                   .reshape(128, T // 128 * BL)
        in2.append({
            "x_t": np.ascontiguousarray(xt),
            "t2": t2_full,
            "blockP": prep["blockP"], "blockPT": prep["blockPT"],
            "colsum": prep["colsum"], "sadj": prep["sadj"],
        })
    res2 = _run(_CACHE["main"], in2, "main")

    # ---- host combine (float64) ----
    lengths = (x != 0).sum(1)
    start64 = start.astype(np.float64)
    end64 = end.astype(np.float64)
    fcb64 = fc_b.astype(np.float64)
    trans64 = trans.astype(np.float64)
    Pe = prep["P_eff"]
    t264 = t2_full.astype(np.float64)
    exp_end = np.exp(end64)

    # numerator: gold-path score, fully vectorized on host
    maskf = (x != 0).astype(np.float64)
    em_tag = t264[x, tags] + fcb64[tags]           # (B,T)
    num = start64[tags[:, 0]] + (em_tag * maskf).sum(1)
    num += (trans64[tags[:, :-1], tags[:, 1:]] * maskf[:, 1:]).sum(1)
    last_tags = tags[np.arange(B), lengths - 1]
    num += end64[last_tags]

    total = 0.0
    for core in range(NCORES):
        r = np.asarray(res2[core]["r_out"], np.float64).reshape(BL, C, S)
        d = np.asarray(res2[core]["d_out"], np.float64).reshape(BL, C, S)
        c = np.einsum('ij,bjs->bis', Pe, d)
        A = np.einsum('bis,bis->bs', r[:, :, :-1], c[:, :, 1:])   # junction s=1..S-1
        Bs = r.sum(axis=1)                                        # (BL, S)
        J = np.log(A) - np.log(Bs[:, 1:])                         # J[:, s-1] <-> junction s
        Jcum = np.concatenate([np.zeros((BL, 1)), np.cumsum(J, axis=1)], axis=1)
        for b in range(BL):
            gb = core * BL + b
            ln = int(lengths[gb])
            sstar = (ln - 1) // L
            logZ = Jcum[b, sstar - 1]        # junctions s=1..sstar-1
            alpha = r[b, :, sstar - 1].copy()
            for t in range(sstar * L, ln):
                w = np.exp(t264[x[gb, t]] + fcb64)
                alpha = (alpha @ Pe) * w
            logZ += np.log(alpha @ exp_end)
            total += -(num[gb] - logZ)
    return np.array(total, dtype=np.float32)


# revision 11
# speedup vs baseline: 7.8950x; 1.1073x over previous
"""CRF negative-log-likelihood kernel for Trainium2 (8 NeuronCores, batch-sharded).

Algorithm:
  - Launch 1 (vocab-sharded): t2 = embedding @ fc_w in bf16 from a
    host-pretransposed embedding (no on-device transposes), fp32 out.
  - Launch 2 (batch-sharded, 8 rows/core): 4 big indirect-DMA gathers
    (8192 offsets each) fetch 16 fp32 per token from t2; PE block-transposes
    to class-on-partition; exp on ACT (bf16 out); segmented linear-space
    forward/backward scan with L=4 steps x S=1024 segments in bf16.
    Tokens are host-permuted k-major so scan slices are contiguous.
  - Host (float64, vectorized): gold-path numerator from t2, rank-1 junction
    chain across segments, exact ragged-tail recompute, final assembly.
"""
import sys
sys.path.insert(0, "/opt/trn_rl_repo")
import numpy as np
import ml_dtypes
from contextlib import ExitStack

import concourse.bass as bass
import concourse.bacc as bacc_mod
import concourse.mybir as mybir
import concourse.tile as tile
from concourse.masks import make_identity
from concourse.bass_utils import run_bass_kernel_spmd

F32 = mybir.dt.float32
BF16 = mybir.dt.bfloat16
I32 = mybir.dt.int32
BF = ml_dtypes.bfloat16

V, E, C = 50257, 128, 16
B, T = 64, 4096
L, S = 4, 1024
VPAD = 51200
VSH = VPAD // 8
BL = 8
NCORES = 8
NG = 4                      # gather instructions in main kernel
GCOL = T // NG              # columns of TM per gather
H = 2                       # independent scan chains (halves of S)
SH = S // H

LAST_EXEC_NS = {}
_TRACE = False
_CACHE = {}
LAST_RESULTS = {}


def build_t2_kernel():
    nc = bacc_mod.Bacc()
    embT = nc.dram_tensor("embT", [E, VSH], BF16, kind="ExternalInput")
    fc_w = nc.dram_tensor("fc_w", [E, C], BF16, kind="ExternalInput")
    t2_s = nc.dram_tensor("t2_s", [VSH, C], F32, kind="ExternalOutput")

    nblk = VSH // 128        # 50 blocks of 128 vocab rows
    NCH = 5                  # input DMA chunks
    CHB = nblk // NCH        # blocks per chunk
    with ExitStack() as ctx:
        tc = ctx.enter_context(tile.TileContext(nc))
        singles = ctx.enter_context(tc.tile_pool(name="singles", bufs=1))
        psum = ctx.enter_context(tc.tile_pool(name="psum", bufs=2, space="PSUM"))

        fcw_sb = singles.tile([E, C], BF16)
        nc.scalar.dma_start(out=fcw_sb[:], in_=fc_w[:])
        EMB = singles.tile([E, VSH], BF16)
        T2 = singles.tile([128, nblk * C], F32)
        for ch in range(NCH):
            c0 = ch * CHB * 128
            eng = nc.sync if ch % 2 == 0 else nc.scalar
            eng.dma_start(out=EMB[:, c0:c0 + CHB * 128],
                          in_=embT[:, c0:c0 + CHB * 128])
        for ch in range(NCH):
            ps = psum.tile([128, CHB * C], F32, tag="ps")
            for i in range(CHB):
                blk = ch * CHB + i
                nc.tensor.matmul(ps[:, i * C:(i + 1) * C],
                                 lhsT=EMB[:, blk * 128:(blk + 1) * 128],
                                 rhs=fcw_sb[:], start=True, stop=True)
            nc.vector.tensor_copy(T2[:, ch * CHB * C:(ch + 1) * CHB * C], ps[:])
        # (128, nblk*C) -> t2_s (VSH, C); dst dims (p, i, j)
        nc.sync.dma_start(
            out=bass.AP(tensor=t2_s[:].tensor, offset=0,
                        ap=[[C, 128], [128 * C, nblk], [1, C]]),
            in_=T2[:])
    return nc


def build_main_kernel():
    nc = bacc_mod.Bacc()
    x_t = nc.dram_tensor("x_t", [128, T // 128 * BL], I32, kind="ExternalInput")
    t2 = nc.dram_tensor("t2", [VPAD, C], F32, kind="ExternalInput")
    blockP = nc.dram_tensor("blockP", [128, 128], BF16, kind="ExternalInput")
    blockPT = nc.dram_tensor("blockPT", [128, 128], BF16, kind="ExternalInput")
    colsum = nc.dram_tensor("colsum", [128, 1], F32, kind="ExternalInput")

    r_out = nc.dram_tensor("r_out", [128, S], BF16, kind="ExternalOutput")
    d_out = nc.dram_tensor("d_out", [128, S], BF16, kind="ExternalOutput")

    with ExitStack() as ctx:
        tc = ctx.enter_context(tile.TileContext(nc))
        singles = ctx.enter_context(tc.tile_pool(name="singles", bufs=1))
        big = ctx.enter_context(tc.tile_pool(name="big", bufs=1))
        psumT = ctx.enter_context(tc.tile_pool(name="psumT", bufs=2, space="PSUM"))
        psumS = ctx.enter_context(tc.tile_pool(name="psumS", bufs=1, space="PSUM"))

        xt_sb = singles.tile([128, T // 128 * BL], I32)
        blockP_sb = singles.tile([128, 128], BF16)
        nc.scalar.dma_start(out=blockP_sb[:], in_=blockP[:])
        blockPT_sb = singles.tile([128, 128], BF16)
        nc.scalar.dma_start(out=blockPT_sb[:], in_=blockPT[:])
        colsum_sb = singles.tile([128, 1], F32)
        nc.scalar.dma_start(out=colsum_sb[:], in_=colsum[:])

        TM = big.tile([128, T], F32)
        EXPG = big.tile([128, T], BF16)
        ident = singles.tile([128, 128], F32)
        make_identity(nc, ident[:])

        # --- gather (4 big indirect DMAs) + PE transpose + exp ---
        # order chosen so the last-arriving chunk leaves short fwd+bwd tails
        NOFF = GCOL // 16        # offsets per gather
        for g in (0, 1, 3, 2):
            nc.sync.dma_start(out=xt_sb[:, g * NOFF:(g + 1) * NOFF],
                              in_=x_t[:, g * NOFF:(g + 1) * NOFF])
            c0 = g * GCOL
            nc.gpsimd.indirect_dma_start(
                out=TM[:, c0:c0 + GCOL],
                out_offset=None,
                in_=t2[:],
                in_offset=bass.IndirectOffsetOnAxis(
                    ap=xt_sb[:, g * NOFF:(g + 1) * NOFF], axis=0),
            )
            nthi = GCOL // 128
            for grp in range(nthi // 8):
                psT = psumT.tile([128, 1024], F32, tag="psT")
                for q in range(8):
                    th = g * nthi + grp * 8 + q
                    nc.tensor.transpose(psT[:, q * 128:(q + 1) * 128],
                                        TM[:, th * 128:(th + 1) * 128], ident[:])
                base = c0 + grp * 1024
                nc.scalar.activation(EXPG[:, base:base + 1024], psT[:],
                                     mybir.ActivationFunctionType.Exp)

        # --- scans: L steps, S segments, split into H independent chains ---
        r_sb = big.tile([128, S], BF16)
        d_sb = big.tile([128, S], BF16)
        for h in range(H):
            a, b = h * SH, (h + 1) * SH
            nc.vector.tensor_scalar_mul(r_sb[:, a:b], EXPG[:, a:b], colsum_sb[:])
            nc.vector.tensor_copy(d_sb[:, a:b],
                                  EXPG[:, (L - 1) * S + a:(L - 1) * S + b])
        for step in range(1, L):
            kf = step
            kb = L - 1 - step
            for h in range(H):
                a, b = h * SH, (h + 1) * SH
                psR = psumS.tile([128, SH], F32, tag=f"psR{h}")
                nc.tensor.matmul(psR[:], lhsT=blockP_sb[:], rhs=r_sb[:, a:b],
                                 start=True, stop=True)
                nc.vector.tensor_mul(r_sb[:, a:b], psR[:],
                                     EXPG[:, kf * S + a:kf * S + b])
                psD = psumS.tile([128, SH], F32, tag=f"psD{h}")
                nc.tensor.matmul(psD[:], lhsT=blockPT_sb[:], rhs=d_sb[:, a:b],
                                 start=True, stop=True)
                nc.vector.tensor_mul(d_sb[:, a:b], psD[:],
                                     EXPG[:, kb * S + a:kb * S + b])

        nc.sync.dma_start(out=r_out[:], in_=r_sb[:])
        nc.scalar.dma_start(out=d_out[:], in_=d_sb[:])
    return nc


def _host_prep(embedding, fc_w, fc_b, trans, start):
    emb_pad = np.zeros((VPAD, E), np.float32)
    emb_pad[:V] = embedding
    embT_pad = np.ascontiguousarray(emb_pad.T).astype(BF)
    P_eff64 = np.exp(trans.astype(np.float64) + fc_b[None, :].astype(np.float64))
    colsum64 = P_eff64.sum(0)
    start_adj = np.exp(start.astype(np.float64) + fc_b) / colsum64
    P_eff32 = P_eff64.astype(np.float32)

    eye8 = np.eye(BL, dtype=np.float32)
    return dict(
        embT_pad=embT_pad,
        P_eff=P_eff64,
        blockP=np.ascontiguousarray(np.kron(eye8, P_eff32)).astype(BF),
        blockPT=np.ascontiguousarray(np.kron(eye8, P_eff32.T.copy())).astype(BF),
        colsum=np.tile(colsum64, BL)[:, None].astype(np.float32),
        log_sadj=np.log(start_adj),
    )


def _run(nc, in_maps, label):
    res = run_bass_kernel_spmd(nc, in_maps, core_ids=list(range(NCORES)),
                               trace=_TRACE)
    if res.exec_time_ns is not None:
        LAST_EXEC_NS[label] = res.exec_time_ns
    LAST_RESULTS[label] = res
    return res.results


def kernel(x, tags, embedding, fc_w, fc_b, start_transitions, end_transitions,
           transitions):
    x = np.asarray(x, np.int32)
    tags = np.asarray(tags, np.int32)
    embedding = np.asarray(embedding, np.float32)
    fc_w = np.asarray(fc_w, np.float32)
    fc_b = np.asarray(fc_b, np.float32)
    trans = np.asarray(transitions, np.float32)
    start = np.asarray(start_transitions, np.float32)
    end = np.asarray(end_transitions, np.float32)

    prep = _host_prep(embedding, fc_w, fc_b, trans, start)

    if "t2" not in _CACHE:
        nc1 = build_t2_kernel()
        nc1.finalize()
        _CACHE["t2"] = nc1
    if "main" not in _CACHE:
        nc2 = build_main_kernel()
        nc2.finalize()
        _CACHE["main"] = nc2

    # ---- launch 1: t2 = emb_pad @ fc_w, vocab-sharded ----
    fcw_bf = fc_w.astype(BF)
    in1 = [{"embT": np.ascontiguousarray(prep["embT_pad"][:, k * VSH:(k + 1) * VSH]),
            "fc_w": fcw_bf} for k in range(NCORES)]
    res1 = _run(_CACHE["t2"], in1, "t2")
    t2_full = np.concatenate([res1[k]["t2_s"] for k in range(NCORES)], axis=0)
    t2_full = np.ascontiguousarray(t2_full, dtype=np.float32)

    # ---- launch 2: main kernel, batch-sharded ----
    # fold sadj into spare t2 rows addressed only by each row's first token
    SPARE = 51100
    t2_full[SPARE:SPARE + B] = t2_full[x[:, 0]] + prep["log_sadj"][None, :]

    # token permutation: G column (th*128+p) holds token t = (col%S)*L + col//S
    cols = np.arange(T)
    tperm = (cols % S) * L + cols // S
    xp = x[:, tperm].copy()
    xp[:, 0] = SPARE + np.arange(B)
    in2 = []
    for k in range(NCORES):
        sl = slice(k * BL, (k + 1) * BL)
        xt = xp[sl].reshape(BL, T // 128, 128).transpose(2, 1, 0) \
                   .reshape(128, T // 128 * BL)
        in2.append({
            "x_t": np.ascontiguousarray(xt),
            "t2": t2_full,
            "blockP": prep["blockP"], "blockPT": prep["blockPT"],
            "colsum": prep["colsum"],
        })
    res2 = _run(_CACHE["main"], in2, "main")

    # ---- host combine (float64) ----
    lengths = (x != 0).sum(1)
    start64 = start.astype(np.float64)
    end64 = end.astype(np.float64)
    fcb64 = fc_b.astype(np.float64)
    trans64 = trans.astype(np.float64)
    Pe = prep["P_eff"]
    t264 = t2_full.astype(np.float64)
    exp_end = np.exp(end64)

    # numerator: gold-path score, fully vectorized on host
    maskf = (x != 0).astype(np.float64)
    em_tag = t264[x, tags] + fcb64[tags]           # (B,T)
    num = start64[tags[:, 0]] + (em_tag * maskf).sum(1)
    num += (trans64[tags[:, :-1], tags[:, 1:]] * maskf[:, 1:]).sum(1)
    last_tags = tags[np.arange(B), lengths - 1]
    num += end64[last_tags]

    total = 0.0
    for core in range(NCORES):
        r = np.asarray(res2[core]["r_out"], np.float64).reshape(BL, C, S)
        d = np.asarray(res2[core]["d_out"], np.float64).reshape(BL, C, S)
        c = np.einsum('ij,bjs->bis', Pe, d)
        A = np.einsum('bis,bis->bs', r[:, :, :-1], c[:, :, 1:])   # junction s=1..S-1
        Bs = r.sum(axis=1)                                        # (BL, S)
        J = np.log(A) - np.log(Bs[:, 1:])                         # J[:, s-1] <-> junction s
        Jcum = np.concatenate([np.zeros((BL, 1)), np.cumsum(J, axis=1)], axis=1)
        for b in range(BL):
            gb = core * BL + b
            ln = int(lengths[gb])
            sstar = (ln - 1) // L
            logZ = Jcum[b, sstar - 1]        # junctions s=1..sstar-1
            alpha = r[b, :, sstar - 1].copy()
            for t in range(sstar * L, ln):
                w = np.exp(t264[x[gb, t]] + fcb64)
                alpha = (alpha @ Pe) * w
            logZ += np.log(alpha @ exp_end)
            total += -(num[gb] - logZ)
    return np.array(total, dtype=np.float32)
